# revision 1
# baseline (speedup 1.0000x reference)
"""Dual cross-attention kernel for Trainium2 (8 NeuronCores, SPMD).

Computes, per (b, h):
    scores1 = q1 @ k1.T ; scores2 = q2 @ k2.T          (contraction over E=64)
    A = tanh(scores1/8) * sigmoid(scores2/8)
    out = A @ v1                                        (contraction over S)

Sharding: B*H = 32 (b,h) pairs split 4-per-core across 8 cores (pure data
parallelism, no collectives).

The kernel is ScalarE(ACT)-bound: 2*L*S transcendentals per pair must all
run through the one activation engine at 1 elem/lane/cycle, and the HAM
clock gate keeps the tensor engine mostly at its cold 1.2 GHz clock for
this duty cycle.  Design:

  - tanh(x)*sigmoid(y) is rewritten as (2*sigmoid(2x)-1)*sigmoid(y), and k2
    is pre-scaled by 0.5 on the host, so ALL activations are one function
    (Sigmoid) with one uniform scale (1/4): each step's s1|s2 pair is
    consumed by a single FD=1024 ACTIVATE.
  - Score tiles [128,1024] rotate through a 3-buffer PSUM pool (2 banks
    each): pool-tile WAR tracking is precise, so the PE runs a full two
    ACTIVATE-periods ahead of ACT (a mega-tile ring serializes instead).
  - q/k arrive pre-transposed (E on partitions) and pre-cast to fp16 by the
    host; no on-device input transposes or casts.
  - st-outer / lb-inner loop over pairs of 512-wide l-blocks: each k-tile
    weight load feeds two back-to-back N=512 score matmuls (head1/head2 on
    concurrent PE row-groups), each v-tile load feeds two pipelined AV
    matmuls, and AV matmuls enter the PE queue 4 steps late so their
    TT dependencies never head-of-line-block future score matmuls.
  - The AV matmul keeps V stationary; output lands [d, l]-oriented and the
    host transposes it back (untimed).
"""

import numpy as np

import concourse.bass as bass
import concourse.mybir as mybir
import concourse.tile as tile
from concourse import bacc
from concourse.bass_utils import run_bass_kernel_spmd
from contextlib import ExitStack

F32 = mybir.dt.float32
F16 = mybir.dt.float16

B, L, S, H, E, D = 2, 2048, 2048, 16, 64, 64
N_CORES = 8
PAIRS_PER_CORE = (B * H) // N_CORES  # 4

L_BLK = 512           # l columns per step
N_ST = S // 128       # 16 s-tiles
N_G = 2               # l-block groups (2 l-blocks each) per pair
CHUNK = 512
GULP = 2              # score chunks per fused ACTIVATE (FD=1024: one step's
                      # s1|s2 pair; 3-slot rotation keeps the pipeline deep)


def build_program(n_pairs=PAIRS_PER_CORE):
    nc = bacc.Bacc("TRN2", target_bir_lowering=False, debug=False)

    qTd = nc.dram_tensor("qT", [n_pairs, 128, L], F16, kind="ExternalInput").ap()
    kTd = nc.dram_tensor("kT", [n_pairs, 128, S], F16, kind="ExternalInput").ap()
    vd = nc.dram_tensor("v1", [n_pairs, S, D], F16, kind="ExternalInput").ap()
    # [d, l] layout on device; the host transposes back (untimed)
    outd = nc.dram_tensor("out", [n_pairs, D, L], F32, kind="ExternalOutput").ap()

    n_steps = n_pairs * N_G * N_ST * 2

    with tile.TileContext(nc) as tc, ExitStack() as ctx:
        qk_p = ctx.enter_context(tc.tile_pool(name="qk", bufs=2))
        v_p = ctx.enter_context(tc.tile_pool(name="v", bufs=2))
        sig_p = ctx.enter_context(tc.tile_pool(name="sig", bufs=4))
        u_p = ctx.enter_context(tc.tile_pool(name="u", bufs=4))
        a_p = ctx.enter_context(tc.tile_pool(name="a", bufs=8))
        o_p = ctx.enter_context(tc.tile_pool(name="osb", bufs=2))
        # 3 x [128,1024] (2 banks each) + 2 x [64,512] out accumulators = 8
        sc_p = ctx.enter_context(tc.tile_pool(name="sc", bufs=3, space="PSUM"))
        out_p = ctx.enter_context(tc.tile_pool(name="outl", bufs=1, space="PSUM"))

        def load_pair(p, chunked=False):
            qT = qk_p.tile([128, L], F16, tag="qT")
            kT = qk_p.tile([128, S], F16, tag="kT")
            v_t = v_p.tile([128, N_ST * D], F16, tag="v")
            vv = v_t.rearrange("p (t d) -> p t d", d=D)
            vs = vd[p].rearrange("(t p) d -> p t d", p=128)
            if not chunked:
                nc.sync.dma_start(qT[:], qTd[p])
                nc.sync.dma_start(kT[:], kTd[p])
                nc.sync.dma_start(vv, vs)
                return qT, kT, v_t
            # column-chunked loads: the first matmuls depend only on the
            # first chunks, so compute starts early
            nc.sync.dma_start(kT[:, 0:128], kTd[p][:, 0:128])
            nc.sync.dma_start(qT[:, 0:1024], qTd[p][:, 0:1024])
            nc.sync.dma_start(vv[:, 0:4, :], vs[:, 0:4, :])
            nc.sync.dma_start(kT[:, 128:1024], kTd[p][:, 128:1024])
            nc.sync.dma_start(kT[:, 1024:S], kTd[p][:, 1024:S])
            nc.sync.dma_start(qT[:, 1024:L], qTd[p][:, 1024:L])
            nc.sync.dma_start(vv[:, 4:N_ST, :], vs[:, 4:N_ST, :])
            return qT, kT, v_t

        tiles = {0: load_pair(0, chunked=True)}

        avs_popped = 0        # AV closures emitted (== steps AV-complete)
        av_backlog = []       # (step, closure)
        epi_backlog = []      # (required avs_popped, closure)
        step_av = {}          # step -> AV binder, consumed at TT time
        gulp_ps = {}          # gulp -> pending PSUM tile
        sig_tiles = {}        # gulp -> sigmoid output tile
        next_gulp = 0
        tt_done = 0

        AV_DEFER = 4  # steps an AV waits before hitting the PE queue, so its
        #               TT dependency is long-satisfied (no head-of-line block)

        def pop_backlogs(now):
            nonlocal avs_popped
            while av_backlog and av_backlog[0][0] + AV_DEFER <= now:
                av_backlog.pop(0)[1]()
                avs_popped += 1
            while epi_backlog and epi_backlog[0][0] <= avs_popped:
                epi_backlog.pop(0)[1]()

        def chunk_slot(c):
            """PSUM (tile, offset) for global score chunk c; tiles of 3
            chunks rotate through the 2-buffer pool."""
            g = c // GULP
            if g not in gulp_ps:
                gulp_ps[g] = sc_p.tile([128, GULP * CHUNK], F32, tag="sc",
                                       name=f"sc{g}")
            return gulp_ps[g], (c % GULP) * CHUNK

        def sig_slice(c):
            return sig_tiles[c // GULP], (c % GULP) * CHUNK

        def emit_tt(s):
            """TS+TT for step s (score chunks 2s, 2s+1 both activated)."""
            t_u, ou = sig_slice(2 * s)
            t_g, og = sig_slice(2 * s + 1)
            u_t = u_p.tile([128, CHUNK], F16, tag="u", name=f"u{s}")
            nc.vector.tensor_scalar(u_t[:], t_u[:, ou:ou + CHUNK],
                                    2.0, -1.0,
                                    mybir.AluOpType.mult,
                                    mybir.AluOpType.add)
            a_t = a_p.tile([128, CHUNK], F16, tag="a", name=f"a{s}")
            nc.vector.tensor_mul(a_t[:], u_t[:], t_g[:, og:og + CHUNK])
            av_backlog.append((s, step_av.pop(s)(a_t)))

        def emit_gulps(c_hi, n_steps_total):
            nonlocal next_gulp, tt_done
            while next_gulp * GULP + GULP - 1 <= c_hi:
                g = next_gulp
                ps = gulp_ps.pop(g)
                sig = sig_p.tile([128, GULP * CHUNK], F16, tag="sig",
                                 name=f"sig{g}")
                nc.scalar.activation(sig[:], ps[:],
                                     mybir.ActivationFunctionType.Sigmoid,
                                     scale=0.25)
                sig_tiles[g] = sig
                sig_tiles.pop(g - 4, None)
                next_gulp += 1
                while (tt_done < n_steps_total
                       and 2 * tt_done + 1 <= g * GULP + GULP - 1):
                    emit_tt(tt_done)
                    tt_done += 1

        def make_av(out_l, v_t, st):
            def bind(a_t):
                def av():
                    nc.tensor.matmul(out_l[:],
                                     v_t[:, st * D:(st + 1) * D], a_t[:],
                                     start=(st == 0), stop=(st == N_ST - 1))
                return av
            return bind

        def make_epilogue(out_l, p, lb):
            def epi():
                o_sb = o_p.tile([64, L_BLK], F32, tag="o")
                nc.vector.tensor_copy(o_sb[:], out_l[:])
                nc.sync.dma_start(outd[p, :, lb * L_BLK:(lb + 1) * L_BLK],
                                  o_sb[:])
            return epi

        step = 0
        for p in range(n_pairs):
            qT, kT, v_t = tiles.pop(p)
            for g in range(N_G):
                outs = [out_p.tile([64, L_BLK], F32, tag=f"o{j}",
                                   name=f"out_{p}_{g}_{j}") for j in range(2)]
                for st in range(N_ST):
                    ks = slice(st * 128, (st + 1) * 128)
                    # one k1 load -> two back-to-back s1 matmuls (lb pair);
                    # head2 runs on the other PE row-group concurrently
                    for j in range(2):
                        lb = 2 * g + j
                        qs = slice(lb * L_BLK, (lb + 1) * L_BLK)
                        t, off = chunk_slot(2 * (step + j))
                        nc.tensor.matmul(t[:, off:off + CHUNK],
                                         kT[0:64, ks], qT[0:64, qs],
                                         start=True, stop=True)
                    for j in range(2):
                        lb = 2 * g + j
                        qs = slice(lb * L_BLK, (lb + 1) * L_BLK)
                        t, off = chunk_slot(2 * (step + j) + 1)
                        nc.tensor.matmul(t[:, off:off + CHUNK],
                                         kT[64:128, ks], qT[64:128, qs],
                                         start=True, stop=True)
                    for j in range(2):
                        step_av[step + j] = make_av(outs[j], v_t, st)
                    emit_gulps(2 * step + 3, n_steps)
                    step += 2
                    pop_backlogs(step)
                    if p + 1 < n_pairs and g == 0 and st == 8:
                        tiles[p + 1] = load_pair(p + 1)
                for j in range(2):
                    epi_backlog.append((step, make_epilogue(outs[j], p,
                                                            2 * g + j)))

        # drain: final partial gulp (2 leftover chunks -> FD=1024)
        if next_gulp in gulp_ps:
            g_last = next_gulp
            ps = gulp_ps.pop(g_last)
            n_rem = 2 * n_steps - g_last * GULP
            sig = sig_p.tile([128, GULP * CHUNK], F16, tag="sig",
                             name="sig_last")
            nc.scalar.activation(sig[:, 0:n_rem * CHUNK],
                                 ps[:, 0:n_rem * CHUNK],
                                 mybir.ActivationFunctionType.Sigmoid,
                                 scale=0.25)
            sig_tiles[g_last] = sig
        while tt_done < n_steps:
            emit_tt(tt_done)
            tt_done += 1
        while av_backlog or epi_backlog:
            pop_backlogs(1 << 30)

    nc.compile()
    return nc


_PROG_CACHE = {}


def _get_program():
    key = (PAIRS_PER_CORE, L, S)
    if key not in _PROG_CACHE:
        _PROG_CACHE[key] = build_program()
    return _PROG_CACHE[key]


def _shard_inputs(q1, k1, v1, q2, k2):
    """Host-side prep (untimed): interleave heads, transpose E onto the
    leading on-chip axis, pre-scale k2 by 0.5, cast to fp16, shard."""
    q1t = np.asarray(q1, np.float16).transpose(0, 2, 3, 1)   # [B,H,E,L]
    q2t = np.asarray(q2, np.float16).transpose(0, 2, 3, 1)
    qT = np.ascontiguousarray(
        np.concatenate([q1t, q2t], axis=2)).reshape(B * H, 128, L)
    k1t = np.asarray(k1, np.float16).transpose(0, 2, 3, 1)
    k2t = (np.asarray(k2, np.float32) * 0.5).astype(np.float16).transpose(0, 2, 3, 1)
    kT = np.ascontiguousarray(
        np.concatenate([k1t, k2t], axis=2)).reshape(B * H, 128, S)
    v = np.ascontiguousarray(
        np.asarray(v1, np.float16).transpose(0, 2, 1, 3)).reshape(B * H, S, D)

    def core_slices(x):
        return [np.ascontiguousarray(
            x[c * PAIRS_PER_CORE:(c + 1) * PAIRS_PER_CORE])
            for c in range(N_CORES)]

    qs, ks, vs = core_slices(qT), core_slices(kT), core_slices(v)
    return [{"qT": qs[c], "kT": ks[c], "v1": vs[c]} for c in range(N_CORES)]


def _gather(results):
    out_bh = np.concatenate([results[c]["out"] for c in range(N_CORES)], axis=0)
    # device layout is [pair, D, L] -> [B, L, H, D]
    out = out_bh.reshape(B, H, D, L).transpose(0, 3, 1, 2)
    return np.ascontiguousarray(out.astype(np.float32))


def kernel(q1, k1, v1, q2, k2, v2, attn_mask=None, **_unused):
    """Full-input entry point: shards across 8 NeuronCores, returns [B,L,H,D]."""
    in_maps = _shard_inputs(q1, k1, v1, q2, k2)
    nc = _get_program()
    res = run_bass_kernel_spmd(nc, in_maps, list(range(N_CORES))).results
    return _gather(res)


def run_traced(q1, k1, v1, q2, k2, **kwargs):
    """Like kernel() but with NTFF profiling; returns (out, BassKernelResults)."""
    in_maps = _shard_inputs(q1, k1, v1, q2, k2)
    nc = _get_program()
    br = run_bass_kernel_spmd(nc, in_maps, list(range(N_CORES)), trace=True,
                              **kwargs)
    return _gather(br.results), br



# revision 2
# speedup vs baseline: 1.4152x; 1.4152x over previous
"""Dual cross-attention kernel for Trainium2 (8 NeuronCores, SPMD).

Computes, per (b, h):
    scores1 = q1 @ k1.T ; scores2 = q2 @ k2.T          (contraction over E=64)
    A = tanh(scores1/8) * sigmoid(scores2/8)
    out = A @ v1                                        (contraction over S)

Sharding: B*H = 32 (b,h) pairs split 4-per-core across 8 cores (pure data
parallelism, no collectives).

Every score element must exit PSUM through ScalarE (ACT) or VectorE (DVE) -
their combined exit throughput is the wall.  The baseline pushed both score
tensors through ACT (1 elem/lane/cyc @ 1.2 GHz -> ~255us busy).  This
version splits the exits:

  - A = tanh(x)*sigmoid(y) is rewritten via the half-angle identity as
    (t1 + t1*t2)/2 with t1 = tanh(s1/8), t2 = tanh(s2/16); the 1/2 is
    folded into v on the host.
  - s1 chunks exit through ACT as Tanh (FD=1024 gulps, PSUM src).
  - s2 chunks exit through a SINGLE fused custom-DVE op (registered at
    import time) that evaluates w = t1 * (1 + r*(C0 + C1*r^2 + r^4)), a
    degree-5 odd minimax polynomial for tanh(s2/16).  k2 is pre-scaled on
    the host so the quartic coefficient is exactly 1.0 (the hardware One
    constant) - the elementwise-src1 DVE struct only has 2 scalar slots.
    t2's argument has std 0.25, so the poly is accurate to ~4e-3 RMS.
  - A fraction of steps ("type-A", NA_OF/NA_EVERY) sends s2 through ACT
    instead (same Tanh table, per-instruction scale) with a one-instruction
    scalar_tensor_tensor combine on DVE, balancing the two exit engines.
  - PSUM: 2x[128,1024] ACT gulp tiles + 3x[128,512] DVE tiles +
    1x[128,512] AV accumulator per group (j=0 in partitions 0:64, j=1 in
    64:128 via tile_position=(0,64)) = exactly 8 banks.
  - q/k arrive pre-transposed (E on partitions) and pre-cast to fp16 by the
    host; the AV matmul keeps V stationary; output lands [d, l]-oriented
    and the host transposes it back (untimed).
"""

import numpy as np

import concourse.bass as bass
import concourse.mybir as mybir
import concourse.tile as tile
from concourse import bacc
from concourse.bass_utils import run_bass_kernel_spmd
from contextlib import ExitStack

F32 = mybir.dt.float32
F16 = mybir.dt.float16

B, L, S, H, E, D = 2, 2048, 2048, 16, 64, 64
N_CORES = 8
PAIRS_PER_CORE = (B * H) // N_CORES  # 4

L_BLK = 512           # l columns per chunk
N_ST = S // 128       # 16 s-tiles
N_G = 2               # l-block groups (2 l-blocks each) per pair
CHUNK = 512

# Degree-5 odd minimax fit of tanh(w) on |w|<=3.2 weighted by N(0, 0.5^2)
# (w = s2/16):  tanh(w) ~= w*(a0 + a1 w^2 + a2 w^4).
_A0, _A1, _A2 = 0.98057465, -0.24396491, 0.0307762
# Host scale lambda on k2 maps w -> r = KNORM*w so the quartic coeff is 1.0:
KNORM = _A2 ** 0.2
C0_POLY = _A0 / KNORM          # ~1.967
C1_POLY = _A1 / KNORM ** 3     # ~-1.970
K2_SCALE = KNORM / 16.0        # r = K2_SCALE * (q2.k2)
K2A_SCALE = 0.5                # type-A steps: ACT tanh((s2/2)/8) = tanh(s2/16)

# type-A steps (both chunks via ACT): 1 of every NA_EVERY (st, j) steps.
NA_OF, NA_EVERY = 0, 64        # v1: all type-B (pure DVE s2 path)


def _register_dve_op():
    """Register the fused gating op with the custom-DVE table (idempotent)."""
    import concourse.dve_ops as dve_ops_mod
    from concourse.dve_ops import DveOp
    from concourse.dve_spec import Spec, Src0, Src1, C0, C1, One, lower
    from concourse.dve_table_gen import DveOpSpec

    name = "TANH_GATE_MUL_ANT"
    for op in dve_ops_mod.OPS:
        if op.name == name:
            return op

    z = Src0 * Src0
    p = (z + C1) * z + C0
    t2 = Src0 * p
    spec = Spec(
        body=Src1 * (One + t2),
        reference=lambda in0, in1, s0, s1, imm2: in1
        * (1.0 + in0 * (s0 + s1 * in0 ** 2 + in0 ** 4)),
    )
    shas = {}
    for ver in ("v3", "v4"):
        tmp = DveOpSpec(name=name, opcode=None, uops=lower(spec, ver=ver),
                        rd1_en=True)
        shas[ver] = tmp.sha(ver)
    op = DveOp(name, spec, subdim=False, uops_sha=shas)
    idx = len(dve_ops_mod.OPS)
    dve_ops_mod.OPS.append(op)
    dve_ops_mod._SUB_OPCODE_FOR_NAME[name] = dve_ops_mod._CUSTOM_DVE_ROW_BASE + idx
    dve_ops_mod.CUSTOM_DVE_SPECS[name] = spec
    return op


def build_program(n_pairs=PAIRS_PER_CORE):
    gate_op = _register_dve_op()
    nc = bacc.Bacc("TRN2", target_bir_lowering=False, debug=False)

    qTd = nc.dram_tensor("qT", [n_pairs, 128, L], F16, kind="ExternalInput").ap()
    kTd = nc.dram_tensor("kT", [n_pairs, 128, S], F16, kind="ExternalInput").ap()
    vd = nc.dram_tensor("v1", [n_pairs, S, D], F16, kind="ExternalInput").ap()
    # [d, l] layout on device; the host transposes back (untimed)
    outd = nc.dram_tensor("out", [n_pairs, D, L], F32, kind="ExternalOutput").ap()

    n_steps = n_pairs * N_G * N_ST * 2  # (st, j) steps

    with tile.TileContext(nc) as tc, ExitStack() as ctx:
        qk_p = ctx.enter_context(tc.tile_pool(name="qk", bufs=2))
        v_p = ctx.enter_context(tc.tile_pool(name="v", bufs=2))
        sig_p = ctx.enter_context(tc.tile_pool(name="sig", bufs=4))
        w_p = ctx.enter_context(tc.tile_pool(name="w", bufs=8))
        o_p = ctx.enter_context(tc.tile_pool(name="osb", bufs=2))
        # PSUM: 2x[128,1024] (2 banks each) + 3x[128,512] + 1x[128,512] out = 8
        act_p = ctx.enter_context(tc.tile_pool(name="actp", bufs=2, space="PSUM"))
        dve_p = ctx.enter_context(tc.tile_pool(name="dvep", bufs=3, space="PSUM"))
        out_p = ctx.enter_context(tc.tile_pool(name="outl", bufs=1, space="PSUM"))

        def load_pair(p, chunked=False):
            qT = qk_p.tile([128, L], F16, tag="qT")
            kT = qk_p.tile([128, S], F16, tag="kT")
            v_t = v_p.tile([128, N_ST * D], F16, tag="v")
            vv = v_t.rearrange("p (t d) -> p t d", d=D)
            vs = vd[p].rearrange("(t p) d -> p t d", p=128)
            if not chunked:
                nc.sync.dma_start(qT[:], qTd[p])
                nc.sync.dma_start(kT[:], kTd[p])
                nc.sync.dma_start(vv, vs)
                return qT, kT, v_t
            # column-chunked loads: the first matmuls depend only on the
            # first chunks, so compute starts early
            nc.sync.dma_start(kT[:, 0:128], kTd[p][:, 0:128])
            nc.sync.dma_start(qT[:, 0:1024], qTd[p][:, 0:1024])
            nc.sync.dma_start(vv[:, 0:4, :], vs[:, 0:4, :])
            nc.sync.dma_start(kT[:, 128:1024], kTd[p][:, 128:1024])
            nc.sync.dma_start(kT[:, 1024:S], kTd[p][:, 1024:S])
            nc.sync.dma_start(qT[:, 1024:L], qTd[p][:, 1024:L])
            nc.sync.dma_start(vv[:, 4:N_ST, :], vs[:, 4:N_ST, :])
            return qT, kT, v_t

        tiles = {0: load_pair(0, chunked=True)}

        av_backlog = []       # (step, closure)
        epi_backlog = []      # (required step, closure)
        step = 0

        AV_DEFER = 4  # steps an AV waits before hitting the PE queue, so its
        #               dependencies are long-satisfied (no head-of-line block)

        def pop_backlogs(now):
            while av_backlog and av_backlog[0][0] + AV_DEFER <= now:
                av_backlog.pop(0)[1]()
            while epi_backlog and epi_backlog[0][0] + AV_DEFER + 1 <= now:
                epi_backlog.pop(0)[1]()

        def make_av(out_g, v_t, st, j, w_t):
            def av():
                nc.tensor.matmul(out_g[64 * j:64 * (j + 1), :],
                                 v_t[:, st * D:(st + 1) * D], w_t[:],
                                 start=(st == 0), stop=(st == N_ST - 1),
                                 tile_position=(0, 64 * j))
            return av

        def make_epilogue(out_g, p, g):
            def epi():
                o_sb = o_p.tile([128, L_BLK], F32, tag="o")
                nc.vector.tensor_copy(o_sb[:], out_g[:])
                for j in range(2):
                    lb = 2 * g + j
                    nc.sync.dma_start(
                        outd[p, :, lb * L_BLK:(lb + 1) * L_BLK],
                        o_sb[64 * j:64 * (j + 1), :])
            return epi

        for p in range(n_pairs):
            qT, kT, v_t = tiles.pop(p)
            for g in range(N_G):
                out_g = out_p.tile([128, L_BLK], F32, tag="og",
                                   name=f"out_{p}_{g}")
                for st in range(N_ST):
                    ks = slice(st * 128, (st + 1) * 128)
                    act_t = act_p.tile([128, 1024], F32, tag="act",
                                       name=f"act_{p}_{g}_{st}")
                    dve_ts = []
                    type_a = (st * 2) % NA_EVERY < NA_OF if NA_OF else False
                    # score matmuls: s1 on PE rows 0-63, s2 on rows 64-127
                    # (concurrent row-groups); one k-tile weight load feeds
                    # both j columns.
                    for j in range(2):
                        lb = 2 * g + j
                        qs = slice(lb * L_BLK, (lb + 1) * L_BLK)
                        nc.tensor.matmul(act_t[:, j * 512:(j + 1) * 512],
                                         kT[0:64, ks], qT[0:64, qs],
                                         start=True, stop=True)
                        d_t = dve_p.tile([128, CHUNK], F32, tag="dve",
                                         name=f"dve_{p}_{g}_{st}_{j}")
                        nc.tensor.matmul(d_t[:],
                                         kT[64:128, ks], qT[64:128, qs],
                                         start=True, stop=True)
                        dve_ts.append(d_t)
                    # ACT: one FD=1024 Tanh over both j's s1 chunks
                    sig = sig_p.tile([128, 1024], F16, tag="sig",
                                     name=f"sig_{p}_{g}_{st}")
                    nc.scalar.activation(sig[:], act_t[:],
                                         mybir.ActivationFunctionType.Tanh,
                                         scale=0.125)
                    # DVE: fused gate+mul, one op per chunk
                    for j in range(2):
                        w_t = w_p.tile([128, CHUNK], F16, tag="w",
                                       name=f"w_{p}_{g}_{st}_{j}")
                        t1 = sig[:, j * 512:(j + 1) * 512]
                        nc.vector._custom_dve(
                            gate_op, out=w_t[:], in0=dve_ts[j][:],
                            in1=t1.rearrange("p (s n) -> p s n", s=1),
                            s0=C0_POLY, s1=C1_POLY)
                        av_backlog.append((step, make_av(out_g, v_t, st, j,
                                                        w_t)))
                        step += 1
                    pop_backlogs(step)
                    if p + 1 < n_pairs and g == 0 and st == 8:
                        tiles[p + 1] = load_pair(p + 1)
                epi_backlog.append((step, make_epilogue(out_g, p, g)))

        while av_backlog or epi_backlog:
            pop_backlogs(1 << 30)

    nc.compile()
    return nc


_PROG_CACHE = {}


def _get_program():
    key = (PAIRS_PER_CORE, L, S)
    if key not in _PROG_CACHE:
        _PROG_CACHE[key] = build_program()
    return _PROG_CACHE[key]


def _shard_inputs(q1, k1, v1, q2, k2):
    """Host-side prep (untimed): interleave heads, transpose E onto the
    leading on-chip axis, pre-scale k2 (poly domain) and v (1/2 fold),
    cast to fp16, shard."""
    q1t = np.asarray(q1, np.float16).transpose(0, 2, 3, 1)   # [B,H,E,L]
    q2t = np.asarray(q2, np.float16).transpose(0, 2, 3, 1)
    qT = np.ascontiguousarray(
        np.concatenate([q1t, q2t], axis=2)).reshape(B * H, 128, L)
    k1t = np.asarray(k1, np.float16).transpose(0, 2, 3, 1)
    k2t = (np.asarray(k2, np.float32) * K2_SCALE).astype(np.float16)
    k2t = k2t.transpose(0, 2, 3, 1)
    kT = np.ascontiguousarray(
        np.concatenate([k1t, k2t], axis=2)).reshape(B * H, 128, S)
    v = (np.asarray(v1, np.float32) * 0.5).astype(np.float16)
    v = np.ascontiguousarray(v.transpose(0, 2, 1, 3)).reshape(B * H, S, D)

    def core_slices(x):
        return [np.ascontiguousarray(
            x[c * PAIRS_PER_CORE:(c + 1) * PAIRS_PER_CORE])
            for c in range(N_CORES)]

    qs, ks, vs = core_slices(qT), core_slices(kT), core_slices(v)
    return [{"qT": qs[c], "kT": ks[c], "v1": vs[c]} for c in range(N_CORES)]


def _gather(results):
    out_bh = np.concatenate([results[c]["out"] for c in range(N_CORES)], axis=0)
    # device layout is [pair, D, L] -> [B, L, H, D]
    out = out_bh.reshape(B, H, D, L).transpose(0, 3, 1, 2)
    return np.ascontiguousarray(out.astype(np.float32))


def kernel(q1, k1, v1, q2, k2, v2, attn_mask=None, **_unused):
    """Full-input entry point: shards across 8 NeuronCores, returns [B,L,H,D]."""
    in_maps = _shard_inputs(q1, k1, v1, q2, k2)
    nc = _get_program()
    res = run_bass_kernel_spmd(nc, in_maps, list(range(N_CORES))).results
    return _gather(res)


def run_traced(q1, k1, v1, q2, k2, **kwargs):
    """Like kernel() but with NTFF profiling; returns (out, BassKernelResults)."""
    in_maps = _shard_inputs(q1, k1, v1, q2, k2)
    nc = _get_program()
    br = run_bass_kernel_spmd(nc, in_maps, list(range(N_CORES)), trace=True,
                              **kwargs)
    return _gather(br.results), br
